# revision 4
# baseline (speedup 1.0000x reference)
"""Trainium2 Bass kernel for BilinearInteraction (v3: x-stationary, e3m4 W moving).

Computes out[b,p,:] = (x[:,pi[p],:] @ W[p]) * x[:,pj[p],:] for all P=276
field pairs (pi,pj) = combinations(24, 2), B=2048, E=128.

Strategy (8 NeuronCores):
  - Data-parallel: shard batch (2048 -> 256 rows/core), replicate W.
  - W is quantized to fp8 e3m4 (4 mantissa bits) on host: measured rel
    error ~1.4e-2 < the 2e-2 gate.  The PE supports mixed-dtype matmul
    (bf16 stationary x e3m4 moving) -- verified bit-exact on HW.  This
    halves the dominant input stream (9 MB -> 4.5 MB per core).
  - Matmul orientation: stationary = x_i^T [E,128 batch] per (group,
    batch-chunk), moving = contiguous W columns at N<=512 -> psum
    [b, p*128+f].  Wide moving segs keep the PE instruction count low
    (~174 LDW+MM pairs); pairs sharing the first field i form column-
    contiguous groups so one stationary serves ~6 x 512-col matmuls.
  - The e3m4 global scale s_W folds into the elementwise multiplier:
    xn = x / s_W on host, out = (xi @ s_W*W) * (xj / s_W).
  - PSUM drain (fp32->bf16 + xj multiply) is split between ScalarE
    (copy, then VectorE bf16 TT at 2x) and a direct VectorE TT from
    PSUM (1x), balanced at ~340/1536 cols direct.
  - All DMA on the single SP HWDGE ring: loads queued up-front
    (interleaved so compute starts ~3us in), stores stream behind
    compute in ~0.79 MB chunks.  Total HBM traffic 25.7 MB/core.
"""

import numpy as np
import ml_dtypes

# ---------------------------------------------------------------- constants
F = 24          # fields
E = 128         # embedding dim
B = 2048        # batch
P = F * (F - 1) // 2        # 276 pairs
NCORES = 8
B_LOCAL = B // NCORES       # 256 rows per core
BCH = 2                     # batch chunks of 128
COLS = P * E                # 35328 output columns per batch chunk
HALF = F * E                # 3072: per-chunk x columns

PAIRS = [(i, j) for i in range(F) for j in range(i + 1, F)]  # p -> (i,j)
GS = [F - 1 - g for g in range(F - 1)]                       # group sizes
GP = [0]
for s in GS:
    GP.append(GP[-1] + s)                                    # pair start per group

BANK = 512                  # fp32 elems per PSUM bank
TCOLS = 3 * BANK            # 1536 cols per psum tile (3 banks)
NT = COLS // TCOLS          # 23 tiles per batch chunk
D_TARGET = 340              # direct-path cols per tile (DVE/ACT balance)

WCHUNK = 32                 # pairs per W load chunk
W_CHUNKS = [(k * WCHUNK, min(P, (k + 1) * WCHUNK))
            for k in range((P + WCHUNK - 1) // WCHUNK)]

SO_TILES = 2                # psum tiles per store chunk (0.79 MB)


def _group_of_col(x):
    p = x // E
    for g in range(len(GS)):
        if GP[g] * E <= p * E < GP[g + 1] * E:
            return g
    raise AssertionError


def _segs(lo, hi):
    """Matmul segs for cols [lo,hi): cut at PSUM banks and group edges.

    Returns (lo, hi, g, start, stop); start/stop are per-bank flags.
    """
    cuts = {lo, hi}
    c = (lo // BANK) * BANK
    while c < hi:
        if lo < c < hi:
            cuts.add(c)
        c += BANK
    for g in range(1, len(GS)):
        e = GP[g] * E
        if lo < e < hi:
            cuts.add(e)
    cuts = sorted(cuts)
    segs = []
    for k in range(len(cuts) - 1):
        a, b = cuts[k], cuts[k + 1]
        assert b - a <= BANK
        segs.append([a, b, _group_of_col(a)])
    out = []
    for k, (a, b, g) in enumerate(segs):
        bank = a // BANK
        first = k == 0 or segs[k - 1][0] // BANK != bank
        last = k == len(segs) - 1 or segs[k + 1][0] // BANK != bank
        out.append((a, b, g, first, last))
    return out


def _runs(lo, hi):
    """Split cols [lo,hi) into maximal runs with a contiguous multiplier.

    Multiplier col for output col x is j(x//128)*128 + x%128 (plus the
    batch-chunk offset added at emission); contiguous across pair edges
    within one group.  Returns (lo, hi, mult_off) triples.
    """
    out = []
    c = lo
    while c < hi:
        g = _group_of_col(c)
        p0 = c // E
        j0 = PAIRS[p0][1]
        moff = j0 * E + (c % E)
        e = min(hi, GP[g + 1] * E)
        out.append((c, e, moff))
        c = e
    return out


def _build_schedule():
    tiles = []
    spent = 0
    for t in range(NT):
        t0 = t * TCOLS
        want = round((t + 1) * D_TARGET)
        d = want - spent
        d += d % 2
        d = max(0, min(TCOLS, d))
        spent += d
        ccols = TCOLS - d
        tiles.append(dict(
            t0=t0,
            ccols=ccols,
            segs=_segs(t0, t0 + TCOLS),
            direct_runs=_runs(t0 + ccols, t0 + TCOLS),
            copied_runs=_runs(t0, t0 + ccols),
        ))
    return tiles


TILES = _build_schedule()

_NC = None


def _build_module():
    global _NC
    if _NC is not None:
        return _NC

    import concourse.bass as bass
    import concourse.tile as tile
    from concourse import bacc, mybir

    bf = mybir.dt.bfloat16
    f8 = mybir.dt.float8e3
    f32 = mybir.dt.float32

    nc = bacc.Bacc("TRN2", target_bir_lowering=False, debug=False)

    # xT[e, c*3072 + f*128 + b]; xn[b, c*3072 + f*128 + e]
    xT = nc.declare_dram_parameter("xT", [E, BCH * HALF], bf, isOutput=False)
    xn = nc.declare_dram_parameter("xn", [E, BCH * HALF], bf, isOutput=False)
    Wt = nc.declare_dram_parameter("Wt", [E, COLS], f8, isOutput=False)
    out = nc.declare_dram_parameter("out", [B_LOCAL, COLS], bf, isOutput=True)

    with tile.TileContext(nc) as tc:
        with (
            tc.tile_pool(name="const", bufs=1) as cpool,
            tc.tile_pool(name="mm", bufs=2) as mmpool,
            tc.tile_pool(name="so", bufs=2) as sopool,
            tc.tile_pool(name="ps", bufs=2, space=bass.MemorySpace.PSUM) as pspool,
        ):
            # ---- loads, queued up-front on the SP ring; ordered so the
            # first matmul can start ~3us in.
            xT_sb = cpool.tile([E, BCH * HALF], bf, tag="xT")
            xn_sb = cpool.tile([E, BCH * HALF], bf, tag="xn")
            w_sb = [cpool.tile([E, (phi - plo) * E], f8, tag=f"w{k}",
                               name=f"w{k}")
                    for k, (plo, phi) in enumerate(W_CHUNKS)]

            def load_w(k):
                plo, phi = W_CHUNKS[k]
                nc.sync.dma_start(out=w_sb[k][:], in_=Wt[:, plo * E: phi * E])

            nc.sync.dma_start(out=xT_sb[:, 0:HALF], in_=xT[:, 0:HALF])
            load_w(0)
            nc.sync.dma_start(out=xn_sb[:, 0:HALF], in_=xn[:, 0:HALF])
            load_w(1)
            nc.sync.dma_start(out=xT_sb[:, HALF:], in_=xT[:, HALF:])
            nc.sync.dma_start(out=xn_sb[:, HALF:], in_=xn[:, HALF:])
            for k in range(2, len(W_CHUNKS)):
                load_w(k)

            # bf16 TTs + stores are deferred until after the NEXT tile's
            # PSUM work is enqueued (drain-priority emission).
            pending = []

            def flush_pending():
                while pending:
                    pending.pop(0)()

            for c in range(BCH):
                so_t = None
                for t, ti in enumerate(TILES):
                    if t % SO_TILES == 0:
                        n_so = min(SO_TILES, NT - t) * TCOLS
                        so_t = sopool.tile([E, n_so], bf, tag="so")
                    so_off = (t % SO_TILES) * TCOLS - ti["t0"]
                    ps = pspool.tile([E, TCOLS], f32, tag="ps")
                    mm_t = mmpool.tile([E, TCOLS], bf, tag="mm")

                    for (a, b, g, first, last) in ti["segs"]:
                        ck = (a // E) // WCHUNK
                        wlo = a - W_CHUNKS[ck][0] * E
                        nc.tensor.matmul(
                            ps[:, a - ti["t0"]: b - ti["t0"]],
                            xT_sb[:, c * HALF + g * E: c * HALF + (g + 1) * E],
                            w_sb[ck][:, wlo: wlo + (b - a)],
                            start=first, stop=last,
                        )
                    if ti["ccols"]:
                        nc.scalar.copy(
                            out=mm_t[:, 0:ti["ccols"]],
                            in_=ps[:, 0:ti["ccols"]],
                        )
                    for (a, b, moff) in ti["direct_runs"]:
                        nc.vector.tensor_mul(
                            so_t[:, so_off + a: so_off + b],
                            ps[:, a - ti["t0"]: b - ti["t0"]],
                            xn_sb[:, c * HALF + moff: c * HALF + moff + (b - a)],
                        )
                    flush_pending()

                    def deferred(c=c, t=t, ti=ti, so_t=so_t, so_off=so_off,
                                 mm_t=mm_t):
                        for (a, b, moff) in ti["copied_runs"]:
                            nc.vector.tensor_mul(
                                so_t[:, so_off + a: so_off + b],
                                mm_t[:, a - ti["t0"]: b - ti["t0"]],
                                xn_sb[:, c * HALF + moff:
                                      c * HALF + moff + (b - a)],
                            )
                        if t % SO_TILES == SO_TILES - 1 or t == NT - 1:
                            clo = (t // SO_TILES) * SO_TILES * TCOLS
                            nc.sync.dma_start(
                                out=out[c * E: (c + 1) * E,
                                        clo: ti["t0"] + TCOLS],
                                in_=so_t[:],
                            )

                    pending.append(deferred)
            flush_pending()

    nc.compile()
    _NC = nc
    return nc


def _prep_inputs(x, W):
    """Host-side shard + relayout + quantize. Returns in_maps for 8 cores."""
    bf = ml_dtypes.bfloat16
    e3 = ml_dtypes.float8_e3m4
    x = np.ascontiguousarray(x, dtype=np.float32)
    W = np.ascontiguousarray(W, dtype=np.float32)

    s_w = 15.0 / float(np.abs(W).max())

    # Wt[e, p*128+f] = W[p,e,f] * s_w   (e3m4)
    Wt = np.ascontiguousarray(
        (W * s_w).transpose(1, 0, 2).reshape(E, COLS)
    ).astype(e3)

    in_maps = []
    for core in range(NCORES):
        xs = x[core * B_LOCAL: (core + 1) * B_LOCAL]      # [256, 24, 128]
        xc = xs.reshape(BCH, E, F, E)                     # [c, b, f, e]
        # xT[e, c*3072 + f*128 + b]
        xTh = np.ascontiguousarray(
            xc.transpose(3, 0, 2, 1).reshape(E, BCH * HALF)
        ).astype(bf)
        # xn[b, c*3072 + f*128 + e], pre-divided by s_w
        xnh = np.ascontiguousarray(
            (xc / s_w).transpose(1, 0, 2, 3).reshape(E, BCH * HALF)
        ).astype(bf)
        in_maps.append({"xT": xTh, "xn": xnh, "Wt": Wt})
    return in_maps


def run_on_hw(x, W, trace=False, **run_kwargs):
    """Run the kernel on the 8 NeuronCores; returns (output fp32, results)."""
    from concourse.bass_utils import run_bass_kernel_spmd

    nc = _build_module()
    in_maps = _prep_inputs(x, W)
    res = run_bass_kernel_spmd(
        nc, in_maps, list(range(NCORES)), trace=trace, **run_kwargs
    )
    shards = []
    for core in range(NCORES):
        o = np.asarray(res.results[core]["out"]).astype(np.float32)
        shards.append(o.reshape(B_LOCAL, P, E))
    return np.ascontiguousarray(np.concatenate(shards, axis=0)), res


def kernel(x, W):
    import os
    try:
        out, _ = run_on_hw(x, W, trace=False)
    except Exception:
        # transient device wedge: retry once with a core reset
        os.environ["NEURON_RT_RESET_CORES"] = "1"
        out, _ = run_on_hw(x, W, trace=False)
    return out


# revision 7
# speedup vs baseline: 1.0023x; 1.0023x over previous
"""Trainium2 Bass kernel for BilinearInteraction (v3: x-stationary, e3m4 W moving).

Computes out[b,p,:] = (x[:,pi[p],:] @ W[p]) * x[:,pj[p],:] for all P=276
field pairs (pi,pj) = combinations(24, 2), B=2048, E=128.

Strategy (8 NeuronCores):
  - Data-parallel: shard batch (2048 -> 256 rows/core), replicate W.
  - W is quantized to fp8 e3m4 (4 mantissa bits) on host: measured rel
    error ~1.4e-2 < the 2e-2 gate.  The PE supports mixed-dtype matmul
    (bf16 stationary x e3m4 moving) -- verified bit-exact on HW.  This
    halves the dominant input stream (9 MB -> 4.5 MB per core).
  - Matmul orientation: stationary = x_i^T [E,128 batch] per (group,
    batch-chunk), moving = contiguous W columns at N<=512 -> psum
    [b, p*128+f].  Wide moving segs keep the PE instruction count low
    (~174 LDW+MM pairs); pairs sharing the first field i form column-
    contiguous groups so one stationary serves ~6 x 512-col matmuls.
  - The e3m4 global scale s_W folds into the elementwise multiplier:
    xn = x / s_W on host, out = (xi @ s_W*W) * (xj / s_W).
  - PSUM drain (fp32->bf16 + xj multiply) is split between ScalarE
    (copy, then VectorE bf16 TT at 2x) and a direct VectorE TT from
    PSUM (1x), balanced at ~340/1536 cols direct.
  - All DMA on the single SP HWDGE ring: loads queued up-front
    (interleaved so compute starts ~3us in), stores stream behind
    compute in ~0.79 MB chunks.  Total HBM traffic 25.7 MB/core.
"""

import numpy as np
import ml_dtypes

# ---------------------------------------------------------------- constants
F = 24          # fields
E = 128         # embedding dim
B = 2048        # batch
P = F * (F - 1) // 2        # 276 pairs
NCORES = 8
B_LOCAL = B // NCORES       # 256 rows per core
BCH = 2                     # batch chunks of 128
COLS = P * E                # 35328 output columns per batch chunk
HALF = F * E                # 3072: per-chunk x columns

PAIRS = [(i, j) for i in range(F) for j in range(i + 1, F)]  # p -> (i,j)
GS = [F - 1 - g for g in range(F - 1)]                       # group sizes
GP = [0]
for s in GS:
    GP.append(GP[-1] + s)                                    # pair start per group

BANK = 512                  # fp32 elems per PSUM bank
TCOLS = 3 * BANK            # 1536 cols per psum tile (3 banks)
NT = COLS // TCOLS          # 23 tiles per batch chunk
# direct-path (VectorE-from-PSUM) region per tile: a whole bank on 2 of
# every 3 tiles (avg 341 cols -> ACT/DVE balance).  Bank-aligned so the
# ScalarE copy of [0,ccols) never waits on the direct bank's matmuls.
D_PATTERN = [BANK, BANK, 0]

WCHUNK = 32                 # pairs per W load chunk
W_CHUNKS = [(k * WCHUNK, min(P, (k + 1) * WCHUNK))
            for k in range((P + WCHUNK - 1) // WCHUNK)]

SO_TILES = 2                # psum tiles per store chunk (0.79 MB)


def _group_of_col(x):
    p = x // E
    for g in range(len(GS)):
        if GP[g] * E <= p * E < GP[g + 1] * E:
            return g
    raise AssertionError


def _segs(lo, hi):
    """Matmul segs for cols [lo,hi): cut at PSUM banks and group edges.

    Returns (lo, hi, g, start, stop); start/stop are per-bank flags.
    """
    cuts = {lo, hi}
    c = (lo // BANK) * BANK
    while c < hi:
        if lo < c < hi:
            cuts.add(c)
        c += BANK
    for g in range(1, len(GS)):
        e = GP[g] * E
        if lo < e < hi:
            cuts.add(e)
    cuts = sorted(cuts)
    segs = []
    for k in range(len(cuts) - 1):
        a, b = cuts[k], cuts[k + 1]
        assert b - a <= BANK
        segs.append([a, b, _group_of_col(a)])
    out = []
    for k, (a, b, g) in enumerate(segs):
        bank = a // BANK
        first = k == 0 or segs[k - 1][0] // BANK != bank
        last = k == len(segs) - 1 or segs[k + 1][0] // BANK != bank
        out.append((a, b, g, first, last))
    return out


def _runs(lo, hi):
    """Split cols [lo,hi) into maximal runs with a contiguous multiplier.

    Multiplier col for output col x is j(x//128)*128 + x%128 (plus the
    batch-chunk offset added at emission); contiguous across pair edges
    within one group.  Returns (lo, hi, mult_off) triples.
    """
    out = []
    c = lo
    while c < hi:
        g = _group_of_col(c)
        p0 = c // E
        j0 = PAIRS[p0][1]
        moff = j0 * E + (c % E)
        e = min(hi, GP[g + 1] * E)
        out.append((c, e, moff))
        c = e
    return out


def _build_schedule():
    tiles = []
    for t in range(NT):
        t0 = t * TCOLS
        d = D_PATTERN[t % len(D_PATTERN)]
        ccols = TCOLS - d
        tiles.append(dict(
            t0=t0,
            ccols=ccols,
            segs=_segs(t0, t0 + TCOLS),
            direct_runs=_runs(t0 + ccols, t0 + TCOLS),
            copied_runs=_runs(t0, t0 + ccols),
        ))
    return tiles


def _dedup_ldweights(nc, mybir):
    """Remove InstLdweights whose weights AP matches the previous LDW on
    the PE stream (the stationary is still loaded); move any syncs onto
    the following instruction.  Verified correct on HW (probe_dedup)."""
    removed = 0
    for f in nc.m.functions:
        for bb in f.blocks:
            insts = bb.instructions
            last_key = None
            to_remove = []
            for idx, inst in enumerate(insts):
                tn = type(inst).__name__
                if tn == 'InstLdweights':
                    key = str(inst.ins[0])
                    if key == last_key:
                        to_remove.append((idx, inst))
                    last_key = key
                elif tn in ('InstMatmult', 'InstMatmultMx', 'InstEventSemaphore',
                            'InstDrain'):
                    pass
                elif getattr(inst, 'engine', None) == mybir.EngineType.PE:
                    last_key = None
            for idx, inst in reversed(to_remove):
                si = inst.sync_info
                if si is not None and (si.on_wait or si.on_update):
                    nxt = insts[idx + 1]
                    nsi = nxt.sync_info
                    if nsi is None:
                        nxt.sync_info = si
                    else:
                        nsi.on_wait.extend(si.on_wait)
                        nsi.on_update.extend(si.on_update)
                        nxt.sync_info = nsi
                insts.remove(inst)
                removed += 1
    return removed


TILES = _build_schedule()

_NC = None


def _build_module():
    global _NC
    if _NC is not None:
        return _NC

    import concourse.bass as bass
    import concourse.tile as tile
    from concourse import bacc, mybir

    bf = mybir.dt.bfloat16
    f8 = mybir.dt.float8e3
    f32 = mybir.dt.float32

    nc = bacc.Bacc("TRN2", target_bir_lowering=False, debug=False)

    # xT[e, c*3072 + f*128 + b]; xn[b, c*3072 + f*128 + e]
    xT = nc.declare_dram_parameter("xT", [E, BCH * HALF], bf, isOutput=False)
    xn = nc.declare_dram_parameter("xn", [E, BCH * HALF], bf, isOutput=False)
    Wt = nc.declare_dram_parameter("Wt", [E, COLS], f8, isOutput=False)
    out = nc.declare_dram_parameter("out", [B_LOCAL, COLS], bf, isOutput=True)

    with tile.TileContext(nc) as tc:
        with (
            tc.tile_pool(name="const", bufs=1) as cpool,
            tc.tile_pool(name="mm", bufs=2) as mmpool,
            tc.tile_pool(name="so", bufs=2) as sopool,
            tc.tile_pool(name="ps", bufs=2, space=bass.MemorySpace.PSUM) as pspool,
        ):
            # ---- loads, queued up-front on the SP ring; ordered so the
            # first matmul can start ~3us in.
            xT_sb = cpool.tile([E, BCH * HALF], bf, tag="xT")
            xn_sb = cpool.tile([E, BCH * HALF], bf, tag="xn")
            w_sb = [cpool.tile([E, (phi - plo) * E], f8, tag=f"w{k}",
                               name=f"w{k}")
                    for k, (plo, phi) in enumerate(W_CHUNKS)]

            def load_w(k):
                plo, phi = W_CHUNKS[k]
                nc.sync.dma_start(out=w_sb[k][:], in_=Wt[:, plo * E: phi * E])

            nc.sync.dma_start(out=xT_sb[:, 0:HALF], in_=xT[:, 0:HALF])
            load_w(0)
            nc.sync.dma_start(out=xn_sb[:, 0:HALF], in_=xn[:, 0:HALF])
            load_w(1)
            nc.sync.dma_start(out=xT_sb[:, HALF:], in_=xT[:, HALF:])
            nc.sync.dma_start(out=xn_sb[:, HALF:], in_=xn[:, HALF:])
            for k in range(2, len(W_CHUNKS)):
                load_w(k)

            # bf16 TTs + stores are deferred until after the NEXT tile's
            # PSUM work is enqueued (drain-priority emission).
            pending = []

            def flush_pending():
                while pending:
                    pending.pop(0)()

            for c in range(BCH):
                so_t = None
                for t, ti in enumerate(TILES):
                    if t % SO_TILES == 0:
                        n_so = min(SO_TILES, NT - t) * TCOLS
                        so_t = sopool.tile([E, n_so], bf, tag="so")
                    so_off = (t % SO_TILES) * TCOLS - ti["t0"]
                    ps = pspool.tile([E, TCOLS], f32, tag="ps")
                    mm_t = mmpool.tile([E, TCOLS], bf, tag="mm")

                    for (a, b, g, first, last) in ti["segs"]:
                        ck = (a // E) // WCHUNK
                        wlo = a - W_CHUNKS[ck][0] * E
                        nc.tensor.matmul(
                            ps[:, a - ti["t0"]: b - ti["t0"]],
                            xT_sb[:, c * HALF + g * E: c * HALF + (g + 1) * E],
                            w_sb[ck][:, wlo: wlo + (b - a)],
                            start=first, stop=last,
                        )
                    if ti["ccols"]:
                        nc.scalar.copy(
                            out=mm_t[:, 0:ti["ccols"]],
                            in_=ps[:, 0:ti["ccols"]],
                        )
                    for (a, b, moff) in ti["direct_runs"]:
                        nc.vector.tensor_mul(
                            so_t[:, so_off + a: so_off + b],
                            ps[:, a - ti["t0"]: b - ti["t0"]],
                            xn_sb[:, c * HALF + moff: c * HALF + moff + (b - a)],
                        )
                    flush_pending()

                    def deferred(c=c, t=t, ti=ti, so_t=so_t, so_off=so_off,
                                 mm_t=mm_t):
                        for (a, b, moff) in ti["copied_runs"]:
                            nc.vector.tensor_mul(
                                so_t[:, so_off + a: so_off + b],
                                mm_t[:, a - ti["t0"]: b - ti["t0"]],
                                xn_sb[:, c * HALF + moff:
                                      c * HALF + moff + (b - a)],
                            )
                        if t % SO_TILES == SO_TILES - 1 or t == NT - 1:
                            clo = (t // SO_TILES) * SO_TILES * TCOLS
                            nc.sync.dma_start(
                                out=out[c * E: (c + 1) * E,
                                        clo: ti["t0"] + TCOLS],
                                in_=so_t[:],
                            )

                    pending.append(deferred)
            flush_pending()

    _dedup_ldweights(nc, mybir)
    nc.compile()
    _NC = nc
    return nc


def _prep_inputs(x, W):
    """Host-side shard + relayout + quantize. Returns in_maps for 8 cores."""
    bf = ml_dtypes.bfloat16
    e3 = ml_dtypes.float8_e3m4
    x = np.ascontiguousarray(x, dtype=np.float32)
    W = np.ascontiguousarray(W, dtype=np.float32)

    s_w = 15.0 / float(np.abs(W).max())

    # Wt[e, p*128+f] = W[p,e,f] * s_w   (e3m4)
    Wt = np.ascontiguousarray(
        (W * s_w).transpose(1, 0, 2).reshape(E, COLS)
    ).astype(e3)

    in_maps = []
    for core in range(NCORES):
        xs = x[core * B_LOCAL: (core + 1) * B_LOCAL]      # [256, 24, 128]
        xc = xs.reshape(BCH, E, F, E)                     # [c, b, f, e]
        # xT[e, c*3072 + f*128 + b]
        xTh = np.ascontiguousarray(
            xc.transpose(3, 0, 2, 1).reshape(E, BCH * HALF)
        ).astype(bf)
        # xn[b, c*3072 + f*128 + e], pre-divided by s_w
        xnh = np.ascontiguousarray(
            (xc / s_w).transpose(1, 0, 2, 3).reshape(E, BCH * HALF)
        ).astype(bf)
        in_maps.append({"xT": xTh, "xn": xnh, "Wt": Wt})
    return in_maps


def run_on_hw(x, W, trace=False, **run_kwargs):
    """Run the kernel on the 8 NeuronCores; returns (output fp32, results)."""
    from concourse.bass_utils import run_bass_kernel_spmd

    nc = _build_module()
    in_maps = _prep_inputs(x, W)
    res = run_bass_kernel_spmd(
        nc, in_maps, list(range(NCORES)), trace=trace, **run_kwargs
    )
    shards = []
    for core in range(NCORES):
        o = np.asarray(res.results[core]["out"]).astype(np.float32)
        shards.append(o.reshape(B_LOCAL, P, E))
    return np.ascontiguousarray(np.concatenate(shards, axis=0)), res


def kernel(x, W):
    import os
    try:
        out, _ = run_on_hw(x, W, trace=False)
    except Exception:
        # transient device wedge: retry once with a core reset
        os.environ["NEURON_RT_RESET_CORES"] = "1"
        out, _ = run_on_hw(x, W, trace=False)
    return out


# revision 13
# speedup vs baseline: 1.0858x; 1.0832x over previous
"""Trainium2 Bass kernel for BilinearInteraction (v3: x-stationary, e3m4 W moving).

Computes out[b,p,:] = (x[:,pi[p],:] @ W[p]) * x[:,pj[p],:] for all P=276
field pairs (pi,pj) = combinations(24, 2), B=2048, E=128.

Strategy (8 NeuronCores):
  - Data-parallel: shard batch (2048 -> 256 rows/core), replicate W.
  - W is quantized to fp8 e3m4 (4 mantissa bits) on host: measured rel
    error ~1.4e-2 < the 2e-2 gate.  The PE supports mixed-dtype matmul
    (bf16 stationary x e3m4 moving) -- verified bit-exact on HW.  This
    halves the dominant input stream (9 MB -> 4.5 MB per core).
  - Matmul orientation: stationary = x_i^T [E,128 batch] per (group,
    batch-chunk), moving = contiguous W columns at N<=512 -> psum
    [b, p*128+f].  Wide moving segs keep the PE instruction count low
    (~174 LDW+MM pairs); pairs sharing the first field i form column-
    contiguous groups so one stationary serves ~6 x 512-col matmuls.
  - The e3m4 global scale s_W folds into the elementwise multiplier:
    xn = x / s_W on host, out = (xi @ s_W*W) * (xj / s_W).
  - PSUM drain (fp32->bf16 + xj multiply) is split between ScalarE
    (copy, then VectorE bf16 TT at 2x) and a direct VectorE TT from
    PSUM (1x), balanced at ~340/1536 cols direct.
  - All DMA on the single SP HWDGE ring: loads queued up-front
    (interleaved so compute starts ~3us in), stores stream behind
    compute in ~0.79 MB chunks.  Total HBM traffic 25.7 MB/core.
"""

import numpy as np
import ml_dtypes

# ---------------------------------------------------------------- constants
F = 24          # fields
E = 128         # embedding dim
B = 2048        # batch
P = F * (F - 1) // 2        # 276 pairs
NCORES = 8
B_LOCAL = B // NCORES       # 256 rows per core
BCH = 2                     # batch chunks of 128
COLS = P * E                # 35328 output columns per batch chunk
HALF = F * E                # 3072: per-chunk x columns

PAIRS = [(i, j) for i in range(F) for j in range(i + 1, F)]  # p -> (i,j)
GS = [F - 1 - g for g in range(F - 1)]                       # group sizes
GP = [0]
for s in GS:
    GP.append(GP[-1] + s)                                    # pair start per group

BANK = 512                  # fp32 elems per PSUM bank
TCOLS = 3 * BANK            # 1536 cols per psum tile (3 banks)
NT = COLS // TCOLS          # 23 tiles per batch chunk
# direct-path (VectorE-from-PSUM) region per tile: a whole bank on 2 of
# every 3 tiles (avg 341 cols -> ACT/DVE balance).  Bank-aligned so the
# ScalarE copy of [0,ccols) never waits on the direct bank's matmuls.
D_PATTERN = [BANK, BANK, 0]
GPS_FRAC = 0.35             # fraction of copied-region multiplies on GpSimd

WCHUNK = 32                 # pairs per W load chunk
W_CHUNKS = [(k * WCHUNK, min(P, (k + 1) * WCHUNK))
            for k in range((P + WCHUNK - 1) // WCHUNK)]

SO_TILES = 2                # psum tiles per store chunk (0.79 MB)


def _group_of_col(x):
    p = x // E
    for g in range(len(GS)):
        if GP[g] * E <= p * E < GP[g + 1] * E:
            return g
    raise AssertionError


def _segs(lo, hi):
    """Matmul segs for cols [lo,hi): cut at PSUM banks and group edges.

    Returns (lo, hi, g, start, stop); start/stop are per-bank flags.
    """
    cuts = {lo, hi}
    c = (lo // BANK) * BANK
    while c < hi:
        if lo < c < hi:
            cuts.add(c)
        c += BANK
    for g in range(1, len(GS)):
        e = GP[g] * E
        if lo < e < hi:
            cuts.add(e)
    cuts = sorted(cuts)
    segs = []
    for k in range(len(cuts) - 1):
        a, b = cuts[k], cuts[k + 1]
        assert b - a <= BANK
        segs.append([a, b, _group_of_col(a)])
    out = []
    for k, (a, b, g) in enumerate(segs):
        bank = a // BANK
        first = k == 0 or segs[k - 1][0] // BANK != bank
        last = k == len(segs) - 1 or segs[k + 1][0] // BANK != bank
        out.append((a, b, g, first, last))
    return out


def _runs(lo, hi):
    """Split cols [lo,hi) into maximal runs with a contiguous multiplier.

    Multiplier col for output col x is j(x//128)*128 + x%128 (plus the
    batch-chunk offset added at emission); contiguous across pair edges
    within one group.  Returns (lo, hi, mult_off) triples.
    """
    out = []
    c = lo
    while c < hi:
        g = _group_of_col(c)
        p0 = c // E
        j0 = PAIRS[p0][1]
        moff = j0 * E + (c % E)
        e = min(hi, GP[g + 1] * E)
        out.append((c, e, moff))
        c = e
    return out


def _build_schedule():
    tiles = []
    gps_spent = 0.0
    ccols_seen = 0
    for t in range(NT):
        t0 = t * TCOLS
        d = D_PATTERN[t % len(D_PATTERN)]
        ccols = TCOLS - d
        # GpSimd takes the tail [s, ccols) of the copied region
        ccols_seen += ccols
        want = GPS_FRAC * ccols_seen
        gcols = int(round((want - gps_spent) / 2)) * 2
        gcols = max(0, min(ccols, gcols))
        gps_spent += gcols
        s = ccols - gcols
        tiles.append(dict(
            t0=t0,
            ccols=ccols,
            segs=_segs(t0, t0 + TCOLS),
            direct_runs=_runs(t0 + ccols, t0 + TCOLS),
            copied_runs=_runs(t0, t0 + s),
            gps_runs=_runs(t0 + s, t0 + ccols),
        ))
    return tiles


def _dedup_ldweights(nc, mybir):
    """Remove InstLdweights whose weights AP matches the previous LDW on
    the PE stream (the stationary is still loaded); move any syncs onto
    the following instruction.  Verified correct on HW (probe_dedup)."""
    removed = 0
    for f in nc.m.functions:
        for bb in f.blocks:
            insts = bb.instructions
            last_key = None
            to_remove = []
            for idx, inst in enumerate(insts):
                tn = type(inst).__name__
                if tn == 'InstLdweights':
                    key = str(inst.ins[0])
                    if key == last_key:
                        to_remove.append((idx, inst))
                    last_key = key
                elif tn in ('InstMatmult', 'InstMatmultMx', 'InstEventSemaphore',
                            'InstDrain'):
                    pass
                elif getattr(inst, 'engine', None) == mybir.EngineType.PE:
                    last_key = None
            for idx, inst in reversed(to_remove):
                si = inst.sync_info
                if si is not None and (si.on_wait or si.on_update):
                    nxt = insts[idx + 1]
                    nsi = nxt.sync_info
                    if nsi is None:
                        nxt.sync_info = si
                    else:
                        nsi.on_wait.extend(si.on_wait)
                        nsi.on_update.extend(si.on_update)
                        nxt.sync_info = nsi
                insts.remove(inst)
                removed += 1
    return removed


TILES = _build_schedule()

_NC = None


def _build_module():
    global _NC
    if _NC is not None:
        return _NC

    import concourse.bass as bass
    import concourse.tile as tile
    from concourse import bacc, mybir

    bf = mybir.dt.bfloat16
    f8 = mybir.dt.float8e3
    f32 = mybir.dt.float32

    nc = bacc.Bacc("TRN2", target_bir_lowering=False, debug=False)

    # xT[e, c*3072 + f*128 + b]; xn[b, c*3072 + f*128 + e]
    xT = nc.declare_dram_parameter("xT", [E, BCH * HALF], bf, isOutput=False)
    xn = nc.declare_dram_parameter("xn", [E, BCH * HALF], bf, isOutput=False)
    Wt = nc.declare_dram_parameter("Wt", [E, COLS], f8, isOutput=False)
    out = nc.declare_dram_parameter("out", [B_LOCAL, COLS], bf, isOutput=True)

    with tile.TileContext(nc) as tc:
        with (
            tc.tile_pool(name="const", bufs=1) as cpool,
            tc.tile_pool(name="mm", bufs=3) as mmpool,
            tc.tile_pool(name="so", bufs=3) as sopool,
            tc.tile_pool(name="ps", bufs=2, space=bass.MemorySpace.PSUM) as pspool,
        ):
            # ---- loads, queued up-front on the SP ring; ordered so the
            # first matmul can start ~3us in.
            xT_sb = cpool.tile([E, BCH * HALF], bf, tag="xT")
            xn_sb = cpool.tile([E, BCH * HALF], bf, tag="xn")
            w_sb = [cpool.tile([E, (phi - plo) * E], f8, tag=f"w{k}",
                               name=f"w{k}")
                    for k, (plo, phi) in enumerate(W_CHUNKS)]

            def load_w(k):
                plo, phi = W_CHUNKS[k]
                nc.sync.dma_start(out=w_sb[k][:], in_=Wt[:, plo * E: phi * E])

            # head loads: just enough for the first tiles; the rest are
            # interleaved with stores inside the loop so the first stores
            # are not stuck behind 18us of queued loads on the FIFO ring.
            nc.sync.dma_start(out=xT_sb[:, 0:HALF], in_=xT[:, 0:HALF])
            load_w(0)
            nc.sync.dma_start(out=xn_sb[:, 0:HALF], in_=xn[:, 0:HALF])
            load_w(1)
            deferred_loads = {
                (0, 1): lambda: load_w(2),
                (0, 3): lambda: load_w(3),
                (0, 5): lambda: load_w(4),
                (0, 7): lambda: load_w(5),
                (0, 9): lambda: nc.sync.dma_start(
                    out=xT_sb[:, HALF:], in_=xT[:, HALF:]),
                (0, 11): lambda: nc.sync.dma_start(
                    out=xn_sb[:, HALF:], in_=xn[:, HALF:]),
                (0, 13): lambda: load_w(6),
                (0, 15): lambda: load_w(7),
                (0, 17): lambda: load_w(8),
            }

            # bf16 TTs + stores are deferred until after the NEXT tile's
            # PSUM work is enqueued (drain-priority emission).
            pending = []

            def flush_pending():
                while pending:
                    pending.pop(0)()

            for c in range(BCH):
                so_t = None
                for t, ti in enumerate(TILES):
                    if t % SO_TILES == 0:
                        n_so = min(SO_TILES, NT - t) * TCOLS
                        so_t = sopool.tile([E, n_so], bf, tag="so")
                    so_off = (t % SO_TILES) * TCOLS - ti["t0"]
                    ps = pspool.tile([E, TCOLS], f32, tag="ps")
                    mm_t = mmpool.tile([E, TCOLS], bf, tag="mm")

                    for (a, b, g, first, last) in ti["segs"]:
                        ck = (a // E) // WCHUNK
                        wlo = a - W_CHUNKS[ck][0] * E
                        nc.tensor.matmul(
                            ps[:, a - ti["t0"]: b - ti["t0"]],
                            xT_sb[:, c * HALF + g * E: c * HALF + (g + 1) * E],
                            w_sb[ck][:, wlo: wlo + (b - a)],
                            start=first, stop=last,
                        )
                    if ti["ccols"]:
                        nc.scalar.copy(
                            out=mm_t[:, 0:ti["ccols"]],
                            in_=ps[:, 0:ti["ccols"]],
                        )
                    for (a, b, moff) in ti["direct_runs"]:
                        nc.vector.tensor_mul(
                            so_t[:, so_off + a: so_off + b],
                            ps[:, a - ti["t0"]: b - ti["t0"]],
                            xn_sb[:, c * HALF + moff: c * HALF + moff + (b - a)],
                        )
                    flush_pending()
                    if (c, t) in deferred_loads:
                        deferred_loads.pop((c, t))()

                    def deferred(c=c, t=t, ti=ti, so_t=so_t, so_off=so_off,
                                 mm_t=mm_t):
                        for (a, b, moff) in ti["gps_runs"]:
                            nc.gpsimd.tensor_mul(
                                so_t[:, so_off + a: so_off + b],
                                mm_t[:, a - ti["t0"]: b - ti["t0"]],
                                xn_sb[:, c * HALF + moff:
                                      c * HALF + moff + (b - a)],
                            )
                        for (a, b, moff) in ti["copied_runs"]:
                            nc.vector.tensor_mul(
                                so_t[:, so_off + a: so_off + b],
                                mm_t[:, a - ti["t0"]: b - ti["t0"]],
                                xn_sb[:, c * HALF + moff:
                                      c * HALF + moff + (b - a)],
                            )
                        if t % SO_TILES == SO_TILES - 1 or t == NT - 1:
                            clo = (t // SO_TILES) * SO_TILES * TCOLS
                            nc.sync.dma_start(
                                out=out[c * E: (c + 1) * E,
                                        clo: ti["t0"] + TCOLS],
                                in_=so_t[:],
                            )

                    pending.append(deferred)
            flush_pending()

    _dedup_ldweights(nc, mybir)
    nc.compile()
    _NC = nc
    return nc


def _prep_inputs(x, W):
    """Host-side shard + relayout + quantize. Returns in_maps for 8 cores."""
    bf = ml_dtypes.bfloat16
    e3 = ml_dtypes.float8_e3m4
    x = np.ascontiguousarray(x, dtype=np.float32)
    W = np.ascontiguousarray(W, dtype=np.float32)

    s_w = 15.0 / float(np.abs(W).max())

    # Wt[e, p*128+f] = W[p,e,f] * s_w   (e3m4)
    Wt = np.ascontiguousarray(
        (W * s_w).transpose(1, 0, 2).reshape(E, COLS)
    ).astype(e3)

    in_maps = []
    for core in range(NCORES):
        xs = x[core * B_LOCAL: (core + 1) * B_LOCAL]      # [256, 24, 128]
        xc = xs.reshape(BCH, E, F, E)                     # [c, b, f, e]
        # xT[e, c*3072 + f*128 + b]
        xTh = np.ascontiguousarray(
            xc.transpose(3, 0, 2, 1).reshape(E, BCH * HALF)
        ).astype(bf)
        # xn[b, c*3072 + f*128 + e], pre-divided by s_w
        xnh = np.ascontiguousarray(
            (xc / s_w).transpose(1, 0, 2, 3).reshape(E, BCH * HALF)
        ).astype(bf)
        in_maps.append({"xT": xTh, "xn": xnh, "Wt": Wt})
    return in_maps


def run_on_hw(x, W, trace=False, **run_kwargs):
    """Run the kernel on the 8 NeuronCores; returns (output fp32, results)."""
    from concourse.bass_utils import run_bass_kernel_spmd

    nc = _build_module()
    in_maps = _prep_inputs(x, W)
    res = run_bass_kernel_spmd(
        nc, in_maps, list(range(NCORES)), trace=trace, **run_kwargs
    )
    shards = []
    for core in range(NCORES):
        o = np.asarray(res.results[core]["out"]).astype(np.float32)
        shards.append(o.reshape(B_LOCAL, P, E))
    return np.ascontiguousarray(np.concatenate(shards, axis=0)), res


def kernel(x, W):
    import os
    try:
        out, _ = run_on_hw(x, W, trace=False)
    except Exception:
        # transient device wedge: retry once with a core reset
        os.environ["NEURON_RT_RESET_CORES"] = "1"
        out, _ = run_on_hw(x, W, trace=False)
    return out


# revision 19
# speedup vs baseline: 1.2018x; 1.1068x over previous
"""Trainium2 Bass kernel for BilinearInteraction (v3: x-stationary, e3m4 W moving).

Computes out[b,p,:] = (x[:,pi[p],:] @ W[p]) * x[:,pj[p],:] for all P=276
field pairs (pi,pj) = combinations(24, 2), B=2048, E=128.

Strategy (8 NeuronCores):
  - Data-parallel: shard batch (2048 -> 256 rows/core), replicate W.
  - W is quantized to fp8 e3m4 (4 mantissa bits) on host: measured rel
    error ~1.4e-2 < the 2e-2 gate.  The PE supports mixed-dtype matmul
    (bf16 stationary x e3m4 moving) -- verified bit-exact on HW.  This
    halves the dominant input stream (9 MB -> 4.5 MB per core).
  - Matmul orientation: stationary = x_i^T [E,128 batch] per (group,
    batch-chunk), moving = contiguous W columns at N<=512 -> psum
    [b, p*128+f].  Wide moving segs keep the PE instruction count low
    (~174 LDW+MM pairs); pairs sharing the first field i form column-
    contiguous groups so one stationary serves ~6 x 512-col matmuls.
  - The e3m4 global scale s_W folds into the elementwise multiplier:
    xn = x / s_W on host, out = (xi @ s_W*W) * (xj / s_W).
  - PSUM drain (fp32->bf16 + xj multiply) is split between ScalarE
    (copy, then VectorE bf16 TT at 2x) and a direct VectorE TT from
    PSUM (1x), balanced at ~340/1536 cols direct.
  - All DMA on the single SP HWDGE ring: loads queued up-front
    (interleaved so compute starts ~3us in), stores stream behind
    compute in ~0.79 MB chunks.  Total HBM traffic 25.7 MB/core.
"""

import numpy as np
import ml_dtypes

# ---------------------------------------------------------------- constants
F = 24          # fields
E = 128         # embedding dim
B = 2048        # batch
P = F * (F - 1) // 2        # 276 pairs
NCORES = 8
B_LOCAL = B // NCORES       # 256 rows per core
BCH = 2                     # batch chunks of 128
COLS = P * E                # 35328 output columns per batch chunk
HALF = F * E                # 3072: per-chunk x columns

PAIRS = [(i, j) for i in range(F) for j in range(i + 1, F)]  # p -> (i,j)
GS = [F - 1 - g for g in range(F - 1)]                       # group sizes
GP = [0]
for s in GS:
    GP.append(GP[-1] + s)                                    # pair start per group

BANK = 512                  # fp32 elems per PSUM bank
TCOLS = 3 * BANK            # 1536 cols per psum tile (3 banks)
NT = COLS // TCOLS          # 23 tiles per batch chunk
# direct-path (VectorE-from-PSUM) region per tile: a whole bank on 2 of
# every 3 tiles (avg 341 cols -> ACT/DVE balance).  Bank-aligned so the
# ScalarE copy of [0,ccols) never waits on the direct bank's matmuls.
D_PATTERN = [BANK, BANK, 0]
GPS_FRAC = 0.0              # GpSimd TT measured ~11x slower than DVE; off
WARM_PRE = 14               # dummy matmuls before the first real one
WARM_PER_TILE = 2           # dummy matmuls appended per tile; keeps the
                            # PE HAM activity monitor at 2.4 GHz (real MM
                            # duty is ~45%, which lets HAM re-throttle)

WCHUNK = 32                 # pairs per W load chunk
W_CHUNKS = [(k * WCHUNK, min(P, (k + 1) * WCHUNK))
            for k in range((P + WCHUNK - 1) // WCHUNK)]

SO_TILES = 2                # psum tiles per store chunk (0.79 MB)


def _group_of_col(x):
    p = x // E
    for g in range(len(GS)):
        if GP[g] * E <= p * E < GP[g + 1] * E:
            return g
    raise AssertionError


def _segs(lo, hi):
    """Matmul segs for cols [lo,hi): cut at PSUM banks and group edges.

    Returns (lo, hi, g, start, stop); start/stop are per-bank flags.
    """
    cuts = {lo, hi}
    c = (lo // BANK) * BANK
    while c < hi:
        if lo < c < hi:
            cuts.add(c)
        c += BANK
    for g in range(1, len(GS)):
        e = GP[g] * E
        if lo < e < hi:
            cuts.add(e)
    cuts = sorted(cuts)
    segs = []
    for k in range(len(cuts) - 1):
        a, b = cuts[k], cuts[k + 1]
        assert b - a <= BANK
        segs.append([a, b, _group_of_col(a)])
    out = []
    for k, (a, b, g) in enumerate(segs):
        bank = a // BANK
        first = k == 0 or segs[k - 1][0] // BANK != bank
        last = k == len(segs) - 1 or segs[k + 1][0] // BANK != bank
        out.append((a, b, g, first, last))
    return out


def _runs(lo, hi):
    """Split cols [lo,hi) into maximal runs with a contiguous multiplier.

    Multiplier col for output col x is j(x//128)*128 + x%128 (plus the
    batch-chunk offset added at emission); contiguous across pair edges
    within one group.  Returns (lo, hi, mult_off) triples.
    """
    out = []
    c = lo
    while c < hi:
        g = _group_of_col(c)
        p0 = c // E
        j0 = PAIRS[p0][1]
        moff = j0 * E + (c % E)
        e = min(hi, GP[g + 1] * E)
        out.append((c, e, moff))
        c = e
    return out


def _build_schedule():
    tiles = []
    gps_spent = 0.0
    ccols_seen = 0
    for t in range(NT):
        t0 = t * TCOLS
        d = D_PATTERN[t % len(D_PATTERN)]
        ccols = TCOLS - d
        # GpSimd takes the tail [s, ccols) of the copied region
        ccols_seen += ccols
        want = GPS_FRAC * ccols_seen
        gcols = int(round((want - gps_spent) / 2)) * 2
        gcols = max(0, min(ccols, gcols))
        gps_spent += gcols
        s = ccols - gcols
        tiles.append(dict(
            t0=t0,
            ccols=ccols,
            segs=_segs(t0, t0 + TCOLS),
            direct_runs=_runs(t0 + ccols, t0 + TCOLS),
            copied_runs=_runs(t0, t0 + s),
            gps_runs=_runs(t0 + s, t0 + ccols),
        ))
    return tiles


def _dedup_ldweights(nc, mybir):
    """Remove InstLdweights whose weights AP matches the previous LDW on
    the PE stream (the stationary is still loaded); move any syncs onto
    the following instruction.  Verified correct on HW (probe_dedup)."""
    removed = 0
    for f in nc.m.functions:
        for bb in f.blocks:
            insts = bb.instructions
            last_key = None
            to_remove = []
            for idx, inst in enumerate(insts):
                tn = type(inst).__name__
                if tn == 'InstLdweights':
                    key = str(inst.ins[0])
                    if key == last_key:
                        to_remove.append((idx, inst))
                    last_key = key
                elif tn in ('InstMatmult', 'InstMatmultMx', 'InstEventSemaphore',
                            'InstDrain'):
                    pass
                elif getattr(inst, 'engine', None) == mybir.EngineType.PE:
                    last_key = None
            for idx, inst in reversed(to_remove):
                si = inst.sync_info
                if si is not None and (si.on_wait or si.on_update):
                    nxt = insts[idx + 1]
                    nsi = nxt.sync_info
                    if nsi is None:
                        nxt.sync_info = si
                    else:
                        nsi.on_wait.extend(si.on_wait)
                        nsi.on_update.extend(si.on_update)
                        nxt.sync_info = nsi
                insts.remove(inst)
                removed += 1
    return removed


TILES = _build_schedule()

_NC = None


def _build_module():
    global _NC
    if _NC is not None:
        return _NC

    import concourse.bass as bass
    import concourse.tile as tile
    from concourse import bacc, mybir

    bf = mybir.dt.bfloat16
    f8 = mybir.dt.float8e3
    f32 = mybir.dt.float32

    nc = bacc.Bacc("TRN2", target_bir_lowering=False, debug=False)

    # xT[e, c*3072 + f*128 + b]; xn[b, c*3072 + f*128 + e]
    xT = nc.declare_dram_parameter("xT", [E, BCH * HALF], bf, isOutput=False)
    xn = nc.declare_dram_parameter("xn", [E, BCH * HALF], bf, isOutput=False)
    Wt = nc.declare_dram_parameter("Wt", [E, COLS], f8, isOutput=False)
    out = nc.declare_dram_parameter("out", [B_LOCAL, COLS], bf, isOutput=True)

    with tile.TileContext(nc) as tc:
        with (
            tc.tile_pool(name="const", bufs=1) as cpool,
            tc.tile_pool(name="mm", bufs=4) as mmpool,
            tc.tile_pool(name="so", bufs=4) as sopool,
            tc.tile_pool(name="ps", bufs=2, space=bass.MemorySpace.PSUM) as pspool,
            tc.tile_pool(name="tr", bufs=1, space=bass.MemorySpace.PSUM) as trpool,
        ):
            # ---- loads, queued up-front on the SP ring; ordered so the
            # first matmul can start ~3us in.
            xT_sb = cpool.tile([E, BCH * HALF], bf, tag="xT")
            xn_sb = cpool.tile([E, BCH * HALF], bf, tag="xn")
            w_sb = [cpool.tile([E, (phi - plo) * E], f8, tag=f"w{k}",
                               name=f"w{k}")
                    for k, (plo, phi) in enumerate(W_CHUNKS)]

            def load_w(k):
                plo, phi = W_CHUNKS[k]
                nc.sync.dma_start(out=w_sb[k][:], in_=Wt[:, plo * E: phi * E])

            # head loads: just enough for the first tiles; the rest are
            # interleaved with stores inside the loop so the first stores
            # are not stuck behind 18us of queued loads on the FIFO ring.
            nc.sync.dma_start(out=xT_sb[:, 0:HALF], in_=xT[:, 0:HALF])
            load_w(0)
            nc.sync.dma_start(out=xn_sb[:, 0:HALF], in_=xn[:, 0:HALF])
            load_w(1)

            # PE warm-up: dummy matmuls into a trash PSUM bank, reading a
            # zeroed SBUF tile.  Keeps HAM's activity window busy so real
            # matmuls run at 2.4 GHz instead of 1.2.
            warm_sb = cpool.tile([E, BANK], bf, tag="warm")
            trash_ps = trpool.tile([E, BANK], f32, tag="trash")
            nc.vector.memzero(warm_sb[:])

            def warm(n):
                for _ in range(n):
                    nc.tensor.matmul(trash_ps[:], warm_sb[:, 0:E],
                                     warm_sb[:], start=True, stop=True)

            warm(WARM_PRE)
            deferred_loads = {
                (0, 1): lambda: load_w(2),
                (0, 3): lambda: load_w(3),
                (0, 5): lambda: load_w(4),
                (0, 7): lambda: load_w(5),
                (0, 9): lambda: nc.sync.dma_start(
                    out=xT_sb[:, HALF:], in_=xT[:, HALF:]),
                (0, 11): lambda: nc.sync.dma_start(
                    out=xn_sb[:, HALF:], in_=xn[:, HALF:]),
                (0, 13): lambda: load_w(6),
                (0, 15): lambda: load_w(7),
                (0, 17): lambda: load_w(8),
            }

            # bf16 TTs + stores are deferred until after the NEXT tile's
            # PSUM work is enqueued (drain-priority emission).
            pending = []

            def flush_pending():
                while pending:
                    pending.pop(0)()

            for c in range(BCH):
                so_t = None
                for t, ti in enumerate(TILES):
                    if t % SO_TILES == 0:
                        n_so = min(SO_TILES, NT - t) * TCOLS
                        so_t = sopool.tile([E, n_so], bf, tag="so")
                    so_off = (t % SO_TILES) * TCOLS - ti["t0"]
                    ps = pspool.tile([E, TCOLS], f32, tag="ps")
                    mm_t = mmpool.tile([E, TCOLS], bf, tag="mm")

                    for (a, b, g, first, last) in ti["segs"]:
                        ck = (a // E) // WCHUNK
                        wlo = a - W_CHUNKS[ck][0] * E
                        nc.tensor.matmul(
                            ps[:, a - ti["t0"]: b - ti["t0"]],
                            xT_sb[:, c * HALF + g * E: c * HALF + (g + 1) * E],
                            w_sb[ck][:, wlo: wlo + (b - a)],
                            start=first, stop=last,
                        )
                    if ti["ccols"]:
                        nc.scalar.copy(
                            out=mm_t[:, 0:ti["ccols"]],
                            in_=ps[:, 0:ti["ccols"]],
                        )
                    for (a, b, moff) in ti["direct_runs"]:
                        nc.vector.tensor_mul(
                            so_t[:, so_off + a: so_off + b],
                            ps[:, a - ti["t0"]: b - ti["t0"]],
                            xn_sb[:, c * HALF + moff: c * HALF + moff + (b - a)],
                        )
                    warm(WARM_PER_TILE)
                    flush_pending()
                    if (c, t) in deferred_loads:
                        deferred_loads.pop((c, t))()

                    def deferred(c=c, t=t, ti=ti, so_t=so_t, so_off=so_off,
                                 mm_t=mm_t):
                        for (a, b, moff) in ti["gps_runs"]:
                            nc.gpsimd.tensor_mul(
                                so_t[:, so_off + a: so_off + b],
                                mm_t[:, a - ti["t0"]: b - ti["t0"]],
                                xn_sb[:, c * HALF + moff:
                                      c * HALF + moff + (b - a)],
                            )
                        for (a, b, moff) in ti["copied_runs"]:
                            nc.vector.tensor_mul(
                                so_t[:, so_off + a: so_off + b],
                                mm_t[:, a - ti["t0"]: b - ti["t0"]],
                                xn_sb[:, c * HALF + moff:
                                      c * HALF + moff + (b - a)],
                            )
                        if t % SO_TILES == SO_TILES - 1 or t == NT - 1:
                            clo = (t // SO_TILES) * SO_TILES * TCOLS
                            nc.sync.dma_start(
                                out=out[c * E: (c + 1) * E,
                                        clo: ti["t0"] + TCOLS],
                                in_=so_t[:],
                            )

                    pending.append(deferred)
            flush_pending()

    _dedup_ldweights(nc, mybir)
    nc.compile()
    _NC = nc
    return nc


def _prep_inputs(x, W):
    """Host-side shard + relayout + quantize. Returns in_maps for 8 cores."""
    bf = ml_dtypes.bfloat16
    e3 = ml_dtypes.float8_e3m4
    x = np.ascontiguousarray(x, dtype=np.float32)
    W = np.ascontiguousarray(W, dtype=np.float32)

    s_w = 15.0 / float(np.abs(W).max())

    # Wt[e, p*128+f] = W[p,e,f] * s_w   (e3m4)
    Wt = np.ascontiguousarray(
        (W * s_w).transpose(1, 0, 2).reshape(E, COLS)
    ).astype(e3)

    in_maps = []
    for core in range(NCORES):
        xs = x[core * B_LOCAL: (core + 1) * B_LOCAL]      # [256, 24, 128]
        xc = xs.reshape(BCH, E, F, E)                     # [c, b, f, e]
        # xT[e, c*3072 + f*128 + b]
        xTh = np.ascontiguousarray(
            xc.transpose(3, 0, 2, 1).reshape(E, BCH * HALF)
        ).astype(bf)
        # xn[b, c*3072 + f*128 + e], pre-divided by s_w
        xnh = np.ascontiguousarray(
            (xc / s_w).transpose(1, 0, 2, 3).reshape(E, BCH * HALF)
        ).astype(bf)
        in_maps.append({"xT": xTh, "xn": xnh, "Wt": Wt})
    return in_maps


def run_on_hw(x, W, trace=False, **run_kwargs):
    """Run the kernel on the 8 NeuronCores; returns (output fp32, results)."""
    from concourse.bass_utils import run_bass_kernel_spmd

    nc = _build_module()
    in_maps = _prep_inputs(x, W)
    res = run_bass_kernel_spmd(
        nc, in_maps, list(range(NCORES)), trace=trace, **run_kwargs
    )
    shards = []
    for core in range(NCORES):
        o = np.asarray(res.results[core]["out"]).astype(np.float32)
        shards.append(o.reshape(B_LOCAL, P, E))
    return np.ascontiguousarray(np.concatenate(shards, axis=0)), res


def kernel(x, W):
    import os
    try:
        out, _ = run_on_hw(x, W, trace=False)
    except Exception:
        # transient device wedge: retry once with a core reset
        os.environ["NEURON_RT_RESET_CORES"] = "1"
        out, _ = run_on_hw(x, W, trace=False)
    return out


# revision 21
# speedup vs baseline: 1.2839x; 1.0684x over previous
"""Trainium2 Bass kernel for BilinearInteraction (v3: x-stationary, e3m4 W moving).

Computes out[b,p,:] = (x[:,pi[p],:] @ W[p]) * x[:,pj[p],:] for all P=276
field pairs (pi,pj) = combinations(24, 2), B=2048, E=128.

Strategy (8 NeuronCores):
  - Data-parallel: shard batch (2048 -> 256 rows/core), replicate W.
  - W is quantized to fp8 e3m4 (4 mantissa bits) on host: measured rel
    error ~1.4e-2 < the 2e-2 gate.  The PE supports mixed-dtype matmul
    (bf16 stationary x e3m4 moving) -- verified bit-exact on HW.  This
    halves the dominant input stream (9 MB -> 4.5 MB per core).
  - Matmul orientation: stationary = x_i^T [E,128 batch] per (group,
    batch-chunk), moving = contiguous W columns at N<=512 -> psum
    [b, p*128+f].  Wide moving segs keep the PE instruction count low
    (~174 LDW+MM pairs); pairs sharing the first field i form column-
    contiguous groups so one stationary serves ~6 x 512-col matmuls.
  - The e3m4 global scale s_W folds into the elementwise multiplier:
    xn = x / s_W on host, out = (xi @ s_W*W) * (xj / s_W).
  - PSUM drain (fp32->bf16 + xj multiply) is split between ScalarE
    (copy, then VectorE bf16 TT at 2x) and a direct VectorE TT from
    PSUM (1x), balanced at ~340/1536 cols direct.
  - All DMA on the single SP HWDGE ring: loads queued up-front
    (interleaved so compute starts ~3us in), stores stream behind
    compute in ~0.79 MB chunks.  Total HBM traffic 25.7 MB/core.
"""

import numpy as np
import ml_dtypes

# ---------------------------------------------------------------- constants
F = 24          # fields
E = 128         # embedding dim
B = 2048        # batch
P = F * (F - 1) // 2        # 276 pairs
NCORES = 8
B_LOCAL = B // NCORES       # 256 rows per core
BCH = 2                     # batch chunks of 128
COLS = P * E                # 35328 output columns per batch chunk
HALF = F * E                # 3072: per-chunk x columns

PAIRS = [(i, j) for i in range(F) for j in range(i + 1, F)]  # p -> (i,j)
GS = [F - 1 - g for g in range(F - 1)]                       # group sizes
GP = [0]
for s in GS:
    GP.append(GP[-1] + s)                                    # pair start per group

BANK = 512                  # fp32 elems per PSUM bank
TCOLS = 3 * BANK            # 1536 cols per psum tile (3 banks)
NT = COLS // TCOLS          # 23 tiles per batch chunk
# direct-path (VectorE-from-PSUM) region per tile: a whole bank on 2 of
# every 3 tiles (avg 341 cols -> ACT/DVE balance).  Bank-aligned so the
# ScalarE copy of [0,ccols) never waits on the direct bank's matmuls.
D_PATTERN = [BANK, BANK, 0]
GPS_FRAC = 0.0              # GpSimd TT measured ~11x slower than DVE; off
WARM_PRE = 14               # dummy matmuls before the first real one
WARM_PER_TILE = 2           # dummy matmuls appended per tile; keeps the
                            # PE HAM activity monitor at 2.4 GHz (real MM
                            # duty is ~45%, which lets HAM re-throttle)

WCHUNK = 32                 # pairs per W load chunk
W_CHUNKS = [(k * WCHUNK, min(P, (k + 1) * WCHUNK))
            for k in range((P + WCHUNK - 1) // WCHUNK)]

SO_TILES = 2                # psum tiles per store chunk (0.79 MB)


def _group_of_col(x):
    p = x // E
    for g in range(len(GS)):
        if GP[g] * E <= p * E < GP[g + 1] * E:
            return g
    raise AssertionError


def _segs(lo, hi):
    """Matmul segs for cols [lo,hi): cut at PSUM banks and group edges.

    Returns (lo, hi, g, start, stop); start/stop are per-bank flags.
    """
    cuts = {lo, hi}
    c = (lo // BANK) * BANK
    while c < hi:
        if lo < c < hi:
            cuts.add(c)
        c += BANK
    for g in range(1, len(GS)):
        e = GP[g] * E
        if lo < e < hi:
            cuts.add(e)
    cuts = sorted(cuts)
    segs = []
    for k in range(len(cuts) - 1):
        a, b = cuts[k], cuts[k + 1]
        assert b - a <= BANK
        segs.append([a, b, _group_of_col(a)])
    out = []
    for k, (a, b, g) in enumerate(segs):
        bank = a // BANK
        first = k == 0 or segs[k - 1][0] // BANK != bank
        last = k == len(segs) - 1 or segs[k + 1][0] // BANK != bank
        out.append((a, b, g, first, last))
    return out


def _runs(lo, hi):
    """Split cols [lo,hi) into maximal runs with a contiguous multiplier.

    Multiplier col for output col x is j(x//128)*128 + x%128 (plus the
    batch-chunk offset added at emission); contiguous across pair edges
    within one group.  Returns (lo, hi, mult_off) triples.
    """
    out = []
    c = lo
    while c < hi:
        g = _group_of_col(c)
        p0 = c // E
        j0 = PAIRS[p0][1]
        moff = j0 * E + (c % E)
        e = min(hi, GP[g + 1] * E)
        out.append((c, e, moff))
        c = e
    return out


def _build_schedule():
    tiles = []
    gps_spent = 0.0
    ccols_seen = 0
    for t in range(NT):
        t0 = t * TCOLS
        d = D_PATTERN[t % len(D_PATTERN)]
        ccols = TCOLS - d
        # GpSimd takes the tail [s, ccols) of the copied region
        ccols_seen += ccols
        want = GPS_FRAC * ccols_seen
        gcols = int(round((want - gps_spent) / 2)) * 2
        gcols = max(0, min(ccols, gcols))
        gps_spent += gcols
        s = ccols - gcols
        tiles.append(dict(
            t0=t0,
            ccols=ccols,
            segs=_segs(t0, t0 + TCOLS),
            direct_runs=_runs(t0 + ccols, t0 + TCOLS),
            copied_runs=_runs(t0, t0 + s),
            gps_runs=_runs(t0 + s, t0 + ccols),
        ))
    return tiles


def _dedup_ldweights(nc, mybir):
    """Remove InstLdweights whose weights AP matches the previous LDW on
    the PE stream (the stationary is still loaded); move any syncs onto
    the following instruction.  Verified correct on HW (probe_dedup)."""
    removed = 0
    for f in nc.m.functions:
        for bb in f.blocks:
            insts = bb.instructions
            last_key = None
            to_remove = []
            for idx, inst in enumerate(insts):
                tn = type(inst).__name__
                if tn == 'InstLdweights':
                    key = str(inst.ins[0])
                    if key == last_key:
                        to_remove.append((idx, inst))
                    last_key = key
                elif tn in ('InstMatmult', 'InstMatmultMx', 'InstEventSemaphore',
                            'InstDrain'):
                    pass
                elif getattr(inst, 'engine', None) == mybir.EngineType.PE:
                    last_key = None
            for idx, inst in reversed(to_remove):
                si = inst.sync_info
                if si is not None and (si.on_wait or si.on_update):
                    nxt = insts[idx + 1]
                    nsi = nxt.sync_info
                    if nsi is None:
                        nxt.sync_info = si
                    else:
                        nsi.on_wait.extend(si.on_wait)
                        nsi.on_update.extend(si.on_update)
                        nxt.sync_info = nsi
                insts.remove(inst)
                removed += 1
    return removed


TILES = _build_schedule()

_NC = None


def _build_module():
    global _NC
    if _NC is not None:
        return _NC

    import concourse.bass as bass
    import concourse.tile as tile
    from concourse import bacc, mybir

    bf = mybir.dt.bfloat16
    f8 = mybir.dt.float8e3
    f32 = mybir.dt.float32

    nc = bacc.Bacc("TRN2", target_bir_lowering=False, debug=False)

    # xT[e, c*3072 + f*128 + b]; xn[b, c*3072 + f*128 + e]
    xT = nc.declare_dram_parameter("xT", [E, BCH * HALF], bf, isOutput=False)
    xn = nc.declare_dram_parameter("xn", [E, BCH * HALF], bf, isOutput=False)
    Wt = nc.declare_dram_parameter("Wt", [E, COLS], f8, isOutput=False)
    out = nc.declare_dram_parameter("out", [B_LOCAL, COLS], bf, isOutput=True)

    with tile.TileContext(nc) as tc:
        with (
            tc.tile_pool(name="const", bufs=1) as cpool,
            tc.tile_pool(name="mm", bufs=6) as mmpool,
            tc.tile_pool(name="so", bufs=6) as sopool,
            tc.tile_pool(name="ps", bufs=2, space=bass.MemorySpace.PSUM) as pspool,
            tc.tile_pool(name="tr", bufs=1, space=bass.MemorySpace.PSUM) as trpool,
        ):
            # ---- loads, queued up-front on the SP ring; ordered so the
            # first matmul can start ~3us in.
            xT_sb = cpool.tile([E, BCH * HALF], bf, tag="xT")
            xn_sb = cpool.tile([E, BCH * HALF], bf, tag="xn")
            w_sb = [cpool.tile([E, (phi - plo) * E], f8, tag=f"w{k}",
                               name=f"w{k}")
                    for k, (plo, phi) in enumerate(W_CHUNKS)]

            def load_w(k):
                plo, phi = W_CHUNKS[k]
                nc.sync.dma_start(out=w_sb[k][:], in_=Wt[:, plo * E: phi * E])

            # head loads: just enough for the first tiles; the rest are
            # interleaved with stores inside the loop so the first stores
            # are not stuck behind 18us of queued loads on the FIFO ring.
            nc.sync.dma_start(out=xT_sb[:, 0:HALF], in_=xT[:, 0:HALF])
            load_w(0)
            nc.sync.dma_start(out=xn_sb[:, 0:HALF], in_=xn[:, 0:HALF])
            load_w(1)

            # PE warm-up: dummy matmuls into a trash PSUM bank, reading a
            # zeroed SBUF tile.  Keeps HAM's activity window busy so real
            # matmuls run at 2.4 GHz instead of 1.2.
            warm_sb = cpool.tile([E, BANK], bf, tag="warm")
            trash_ps = trpool.tile([E, BANK], f32, tag="trash")
            nc.vector.memzero(warm_sb[:])

            def warm(n):
                for _ in range(n):
                    nc.tensor.matmul(trash_ps[:], warm_sb[:, 0:E],
                                     warm_sb[:], start=True, stop=True)

            warm(WARM_PRE)
            deferred_loads = {
                (0, 1): lambda: load_w(2),
                (0, 3): lambda: load_w(3),
                (0, 5): lambda: load_w(4),
                (0, 7): lambda: load_w(5),
                (0, 9): lambda: load_w(6),
                (0, 11): lambda: load_w(7),
                (0, 13): lambda: load_w(8),
                (0, 15): lambda: nc.sync.dma_start(
                    out=xT_sb[:, HALF:], in_=xT[:, HALF:]),
                (0, 17): lambda: nc.sync.dma_start(
                    out=xn_sb[:, HALF:], in_=xn[:, HALF:]),
            }

            # bf16 TTs + stores are deferred until after the NEXT tile's
            # PSUM work is enqueued (drain-priority emission).
            pending = []

            def flush_pending():
                while pending:
                    pending.pop(0)()

            for c in range(BCH):
                so_t = None
                for t, ti in enumerate(TILES):
                    if t % SO_TILES == 0:
                        n_so = min(SO_TILES, NT - t) * TCOLS
                        so_t = sopool.tile([E, n_so], bf, tag="so")
                    so_off = (t % SO_TILES) * TCOLS - ti["t0"]
                    ps = pspool.tile([E, TCOLS], f32, tag="ps")
                    mm_t = mmpool.tile([E, TCOLS], bf, tag="mm")

                    for (a, b, g, first, last) in ti["segs"]:
                        ck = (a // E) // WCHUNK
                        wlo = a - W_CHUNKS[ck][0] * E
                        nc.tensor.matmul(
                            ps[:, a - ti["t0"]: b - ti["t0"]],
                            xT_sb[:, c * HALF + g * E: c * HALF + (g + 1) * E],
                            w_sb[ck][:, wlo: wlo + (b - a)],
                            start=first, stop=last,
                        )
                    if ti["ccols"]:
                        nc.scalar.copy(
                            out=mm_t[:, 0:ti["ccols"]],
                            in_=ps[:, 0:ti["ccols"]],
                        )
                    for (a, b, moff) in ti["direct_runs"]:
                        nc.vector.tensor_mul(
                            so_t[:, so_off + a: so_off + b],
                            ps[:, a - ti["t0"]: b - ti["t0"]],
                            xn_sb[:, c * HALF + moff: c * HALF + moff + (b - a)],
                        )
                    warm(WARM_PER_TILE)
                    flush_pending()
                    if (c, t) in deferred_loads:
                        deferred_loads.pop((c, t))()

                    def deferred(c=c, t=t, ti=ti, so_t=so_t, so_off=so_off,
                                 mm_t=mm_t):
                        for (a, b, moff) in ti["gps_runs"]:
                            nc.gpsimd.tensor_mul(
                                so_t[:, so_off + a: so_off + b],
                                mm_t[:, a - ti["t0"]: b - ti["t0"]],
                                xn_sb[:, c * HALF + moff:
                                      c * HALF + moff + (b - a)],
                            )
                        for (a, b, moff) in ti["copied_runs"]:
                            nc.vector.tensor_mul(
                                so_t[:, so_off + a: so_off + b],
                                mm_t[:, a - ti["t0"]: b - ti["t0"]],
                                xn_sb[:, c * HALF + moff:
                                      c * HALF + moff + (b - a)],
                            )
                        if t % SO_TILES == SO_TILES - 1 or t == NT - 1:
                            clo = (t // SO_TILES) * SO_TILES * TCOLS
                            nc.sync.dma_start(
                                out=out[c * E: (c + 1) * E,
                                        clo: ti["t0"] + TCOLS],
                                in_=so_t[:],
                            )

                    pending.append(deferred)
            flush_pending()

    _dedup_ldweights(nc, mybir)
    nc.compile()
    _NC = nc
    return nc


def _prep_inputs(x, W):
    """Host-side shard + relayout + quantize. Returns in_maps for 8 cores."""
    bf = ml_dtypes.bfloat16
    e3 = ml_dtypes.float8_e3m4
    x = np.ascontiguousarray(x, dtype=np.float32)
    W = np.ascontiguousarray(W, dtype=np.float32)

    s_w = 15.0 / float(np.abs(W).max())

    # Wt[e, p*128+f] = W[p,e,f] * s_w   (e3m4)
    Wt = np.ascontiguousarray(
        (W * s_w).transpose(1, 0, 2).reshape(E, COLS)
    ).astype(e3)

    in_maps = []
    for core in range(NCORES):
        xs = x[core * B_LOCAL: (core + 1) * B_LOCAL]      # [256, 24, 128]
        xc = xs.reshape(BCH, E, F, E)                     # [c, b, f, e]
        # xT[e, c*3072 + f*128 + b]
        xTh = np.ascontiguousarray(
            xc.transpose(3, 0, 2, 1).reshape(E, BCH * HALF)
        ).astype(bf)
        # xn[b, c*3072 + f*128 + e], pre-divided by s_w
        xnh = np.ascontiguousarray(
            (xc / s_w).transpose(1, 0, 2, 3).reshape(E, BCH * HALF)
        ).astype(bf)
        in_maps.append({"xT": xTh, "xn": xnh, "Wt": Wt})
    return in_maps


def run_on_hw(x, W, trace=False, **run_kwargs):
    """Run the kernel on the 8 NeuronCores; returns (output fp32, results)."""
    from concourse.bass_utils import run_bass_kernel_spmd

    nc = _build_module()
    in_maps = _prep_inputs(x, W)
    res = run_bass_kernel_spmd(
        nc, in_maps, list(range(NCORES)), trace=trace, **run_kwargs
    )
    shards = []
    for core in range(NCORES):
        o = np.asarray(res.results[core]["out"]).astype(np.float32)
        shards.append(o.reshape(B_LOCAL, P, E))
    return np.ascontiguousarray(np.concatenate(shards, axis=0)), res


def kernel(x, W):
    import os
    try:
        out, _ = run_on_hw(x, W, trace=False)
    except Exception:
        # transient device wedge: retry once with a core reset
        os.environ["NEURON_RT_RESET_CORES"] = "1"
        out, _ = run_on_hw(x, W, trace=False)
    return out
